# revision 5
# baseline (speedup 1.0000x reference)
"""Trainium2 Bass kernel for Mixtral-style GQA attention (v2, bf16).

Full module: y = Attn(RoPE(hs@Wq), RoPE(hs@Wk), hs@Wv) @ Wo
  T=2048, HIDDEN=4096, 32 Q heads / 8 KV heads, head_dim=128, causal,
  neox rotate-half RoPE (base 1e6), fp32 in/out.

Sharding (8 cores, tensor-parallel over heads):
  core c: Q heads 4c..4c+3 (Wq cols c*512:+512), KV head c (Wk/Wv cols
  c*128:+128), Wo rows c*512:+512.  Each core computes a partial
  y^T [4096, 2048] (bf16); host sums the 8 partials and transposes.

v2 design (vs v1): everything bf16 on the PE; hidden_states transposed
and weights re-laid-out on the HOST so the device does zero transposes
and all DMA is contiguous; RoPE sin/cos tables precomputed on host; V
projected directly into natural [t,d] layout; causal diagonal S blocks
column-trimmed; one continuous PE stream: attention for q-column 0 is
pre-computed inside the last projection group, and out-projection of
column g-1 is interleaved into attention column g as latency filler.
"""
import math
import os

import numpy as np

import concourse.bass as bass
import concourse.mybir as mybir
import concourse.tile as tile
from concourse import bacc
from concourse.bass_utils import run_bass_kernel_spmd

F32 = mybir.dt.float32
F32R = mybir.dt.float32r
BF16 = mybir.dt.bfloat16
I32 = mybir.dt.int32
AF = mybir.ActivationFunctionType
ALU = mybir.AluOpType

T = 2048
HID = 4096
NH = 4            # q heads per core
D = 128           # head dim
DQ = NH * D       # 512
G = 512           # seq group size
NG = T // G       # 4
KT = HID // 128   # 32 hidden k-tiles
NJ = T // 128     # 16 key blocks
NCORES = 8

SCALE = 1.0 / math.sqrt(D)

LAST_EXEC_NS = None


def _emit(nc):
    # host-prepared layouts (see kernel() below)
    hst = nc.dram_tensor("hst", [NG, KT, 128, G], BF16, kind="ExternalInput").ap()
    wq = nc.dram_tensor("wq", [128, KT, DQ], BF16, kind="ExternalInput").ap()
    wk = nc.dram_tensor("wk", [128, KT, D], BF16, kind="ExternalInput").ap()
    wv = nc.dram_tensor("wv", [128, KT, D], BF16, kind="ExternalInput").ap()
    wo = nc.dram_tensor("wo", [128, NH, HID], BF16, kind="ExternalInput").ap()
    cosd = nc.dram_tensor("cosd", [128, T], BF16, kind="ExternalInput").ap()
    sind = nc.dram_tensor("sind", [128, T], BF16, kind="ExternalInput").ap()
    yt = nc.dram_tensor("yt", [KT, NG, 128, G], BF16, kind="ExternalOutput").ap()

    with tile.TileContext(nc) as tc:
        with (
            tc.tile_pool(name="const", bufs=1) as const,
            tc.tile_pool(name="res", bufs=1) as res,
            tc.tile_pool(name="hp", bufs=8) as hp,
            tc.tile_pool(name="ro", bufs=2) as ro,
            tc.tile_pool(name="e0p", bufs=16) as e0p,
            tc.tile_pool(name="ex", bufs=6) as ex,
            tc.tile_pool(name="sc", bufs=2) as sc,
            tc.tile_pool(name="yo", bufs=4) as yo,
        ):
            # ---------------- constants ----------------
            onesf = const.tile([128, 1], F32, name="onesf", tag="onesf")
            nc.gpsimd.memset(onesf[:], 1.0)
            ones = const.tile([128, 1], F32R, name="ones", tag="ones")
            nc.scalar.copy(ones[:], onesf[:])

            # resident weights / tables / activations
            wq_sb = res.tile([128, KT, DQ], BF16, name="wq_sb", tag="wq_sb")
            wk_sb = res.tile([128, KT, D], BF16, name="wk_sb", tag="wk_sb")
            wv_sb = res.tile([128, KT, D], BF16, name="wv_sb", tag="wv_sb")
            wo_sb = res.tile([128, NH, HID], BF16, name="wo_sb", tag="wo_sb")
            cosf = res.tile([128, T], BF16, name="cosf", tag="cosf")
            sinpm = res.tile([128, T], BF16, name="sinpm", tag="sinpm")
            qt = [res.tile([128, T], BF16, name=f"qt{h}", tag=f"qt{h}")
                  for h in range(NH)]
            kt = res.tile([128, T], BF16, name="kt", tag="kt")
            vnat = res.tile([128, NJ, D], BF16, name="vnat", tag="vnat")

            # pre-computed exp tiles for attention column g=0 (filled
            # during projection group s=3, consumed right after)
            a0_es = [None] * (NH * 4)

            def rope(src_ps, dst, ssl, use_dve):
                raw = ro.tile([128, G], BF16, name="raw", tag="raw", bufs=3)
                if use_dve:
                    nc.vector.tensor_copy(raw[:], src_ps[:])
                else:
                    nc.scalar.copy(raw[:], src_ps[:])
                rot = ro.tile([128, G], BF16, name="rot", tag="rot", bufs=3)
                nc.scalar.dma_start(rot[0:64, :], raw[64:128, :])
                nc.scalar.dma_start(rot[64:128, :], raw[0:64, :])
                tmp = ro.tile([128, G], BF16, name="tmp", tag="tmp", bufs=2)
                nc.vector.tensor_mul(tmp[:], rot[:], sinpm[:, ssl])
                nc.vector.tensor_mul(dst, raw[:], cosf[:, ssl])
                nc.vector.tensor_add(dst, dst, tmp[:])

            def emit_exp(e, s_ps, g, j):
                """exp + causal handling for S^T block j of column g."""
                r = j - 4 * g
                c0 = 128 * r if r > 0 else 0
                if c0 > 0:
                    nc.gpsimd.memset(e[:, 0:c0], 0.0)
                nc.scalar.activation(e[:, c0:G], s_ps[:, c0:G], AF.Exp,
                                     scale=SCALE)
                if r >= 0:
                    nc.gpsimd.affine_select(
                        out=e[:, c0:G], in_=e[:, c0:G],
                        compare_op=ALU.is_ge, fill=0.0,
                        base=0, channel_multiplier=-1,
                        pattern=[[1, G - c0]])
                return c0

            # ---------------- phase P: projections ----------------
            with (
                tc.tile_pool(name="accp", bufs=6, space="PSUM") as accp,
                tc.tile_pool(name="pre", bufs=2, space="PSUM") as pre,
            ):
                for s in range(NG):
                    ssl = bass.ts(s, G)
                    q_ps = [accp.tile([128, G], F32, name=f"qps{f}",
                                      tag="acc") for f in range(NH)]
                    k_ps = accp.tile([128, G], F32, name="kps", tag="acc")
                    v_ps = accp.tile([128, NH, D], F32, name="vps", tag="acc")

                    for k in range(KT):
                        ht = hp.tile([128, G], BF16, name="ht", tag="ht")
                        nc.sync.dma_start(ht[:], hst[s, k])
                        # stream weights during s=0, misc during s=1
                        if s == 0:
                            if k % 8 == 0:
                                ck = bass.ds(k, 8)
                                nc.sync.dma_start(wq_sb[:, ck, :], wq[:, ck, :])
                                nc.sync.dma_start(wk_sb[:, ck, :], wk[:, ck, :])
                                nc.sync.dma_start(wv_sb[:, ck, :], wv[:, ck, :])
                            if k == 9:
                                nc.sync.dma_start(cosf[:], cosd)
                            if k == 11:
                                nc.sync.dma_start(sinpm[:], sind)
                        elif s == 1 and k < NH:
                            nc.sync.dma_start(wo_sb[:, k, :], wo[:, k, :])
                        st = (k == 0)
                        sp = (k == KT - 1)
                        for f in range(NH):
                            nc.tensor.matmul(
                                q_ps[f][:], wq_sb[:, k, f * 128:(f + 1) * 128],
                                ht[:], start=st, stop=sp)
                        nc.tensor.matmul(k_ps[:], wk_sb[:, k, :], ht[:],
                                         start=st, stop=sp)
                        # all 4 tb regions live in ONE 2KB psum bank and
                        # start=True zeroes the whole bank: only the first
                        # write starts it, only the last one stops it.
                        for tb in range(NH):
                            nc.tensor.matmul(
                                v_ps[:, tb, :], ht[:, tb * 128:(tb + 1) * 128],
                                wv_sb[:, k, :], start=(st and tb == 0),
                                stop=(sp and tb == NH - 1))

                        # pre-compute attention for q-column 0 during s=3
                        if s == 3 and k >= KT - 16:
                            i = k - (KT - 16)
                            h, r = divmod(i, 4)
                            s_ps = pre.tile([128, G], F32, name="sps",
                                            tag="sps")
                            c0 = 128 * r
                            nc.tensor.matmul(
                                s_ps[:, c0:G], kt[:, r * 128:(r + 1) * 128],
                                qt[h][:, c0:G], start=True, stop=True)
                            e = e0p.tile([128, G], BF16, name="e0", tag="e0")
                            emit_exp(e, s_ps, 0, r)
                            a0_es[i] = e

                    # epilogue: RoPE for q heads + k; v to natural layout
                    for f in range(NH):
                        rope(q_ps[f], qt[f][:, ssl], ssl, use_dve=(f % 2 == 1))
                    rope(k_ps, kt[:, ssl], ssl, use_dve=False)
                    nc.vector.tensor_copy(vnat[:, NH * s:NH * s + NH, :],
                                          v_ps[:])

            # ---------------- phase A + O: attention & out-proj ----------
            with (
                tc.tile_pool(name="pss", bufs=3, space="PSUM") as pss,
                tc.tile_pool(name="pso", bufs=2, space="PSUM") as pso,
                tc.tile_pool(name="pssum", bufs=1, space="PSUM") as pssum,
                tc.tile_pool(name="psy", bufs=2, space="PSUM") as psy,
            ):
                def head_tail(h, g, o_ps, sumacc):
                    """softmax-normalize accumulated PV -> qt[h] (as O^T)."""
                    gsl = bass.ts(g, G)
                    s_sum = pssum.tile([1, G], F32, name="ssum", tag="ssum")
                    nc.tensor.matmul(s_sum[:], ones[:], sumacc[:],
                                     start=True, stop=True)
                    recrow = sc.tile([1, G], F32, name="recrow", tag="recrow")
                    nc.vector.reciprocal(recrow[:], s_sum[:])
                    recb = sc.tile([128, G], F32, name="recb", tag="recb")
                    nc.gpsimd.partition_broadcast(recb[:], recrow[:])
                    nc.vector.tensor_mul(qt[h][:, gsl], o_ps[:], recb[:])

                def m_item(g_out, m):
                    """out-projection tile m for q-column g_out."""
                    gsl = bass.ts(g_out, G)
                    y_ps = psy.tile([128, G], F32, name="yps", tag="yps")
                    for f in range(NH):
                        nc.tensor.matmul(y_ps[:], wo_sb[:, f, m * 128:(m + 1) * 128],
                                         qt[f][:, gsl],
                                         start=(f == 0), stop=(f == NH - 1))
                    y_sb = yo.tile([128, G], BF16, name="ysb", tag="ysb")
                    if m % 2 == 0:
                        nc.vector.tensor_copy(y_sb[:], y_ps[:])
                    else:
                        nc.scalar.copy(y_sb[:], y_ps[:])
                    nc.sync.dma_start(yt[m, g_out], y_sb[:])

                # deferred softmax-tail: emit each head's tail after the
                # next head's PE stream is primed, so the tiny ssum matmul
                # (which waits on the DVE sum chain) never stalls the PE.
                pending = []

                def flush_tail():
                    while pending:
                        head_tail(*pending.pop(0))

                # ---- column 0: PV of the pre-computed exps (pure stream)
                for h in range(NH):
                    sumacc = sc.tile([128, G], F32R, name="sumacc",
                                     tag="sumacc", bufs=2)
                    o_ps = pso.tile([128, G], F32, name="ops", tag="ops")
                    for j in range(4):
                        e = a0_es[h * 4 + j]
                        c0 = 128 * j
                        if j == 0:
                            nc.vector.tensor_copy(sumacc[:], e[:])
                        else:
                            nc.vector.tensor_add(sumacc[:, c0:G],
                                                 sumacc[:, c0:G], e[:, c0:G])
                        nc.tensor.matmul(o_ps[:], vnat[:, j, :], e[:],
                                         start=(j == 0), stop=(j == 3))
                        if j == 1:
                            flush_tail()
                    pending.append((h, 0, o_ps, sumacc))

                # ---- columns 1..3 with out-proj of g-1 as filler
                for g in range(1, NG):
                    jn = 4 * g + 4
                    mq = list(range(KT))  # out-proj tiles of column g-1

                    def fill(n, g=g, mq=mq):
                        for _ in range(n):
                            if mq:
                                m_item(g - 1, mq.pop(0))

                    for h in range(NH):
                        s_tiles = {}

                        def emit_s(j, h=h, g=g):
                            s_ps = pss.tile([128, G], F32, name="sps",
                                            tag="sps")
                            r = j - 4 * g
                            c0 = 128 * r if r > 0 else 0
                            nc.tensor.matmul(
                                s_ps[:, c0:G], kt[:, j * 128:(j + 1) * 128],
                                qt[h][:, bass.ds(g * G + c0, G - c0)],
                                start=True, stop=True)
                            s_tiles[j] = s_ps

                        for j in range(3):
                            emit_s(j)
                        flush_tail()
                        fill(2)
                        sumacc = sc.tile([128, G], F32R, name="sumacc",
                                         tag="sumacc", bufs=2)
                        o_ps = pso.tile([128, G], F32, name="ops", tag="ops")
                        for j in range(jn):
                            s_ps = s_tiles.pop(j)
                            e = ex.tile([128, G], BF16, name="esb", tag="esb")
                            c0 = emit_exp(e, s_ps, g, j)
                            if j + 3 < jn:
                                emit_s(j + 3)
                            if j == 0:
                                nc.vector.tensor_copy(sumacc[:], e[:])
                            else:
                                nc.vector.tensor_add(sumacc[:, c0:G],
                                                     sumacc[:, c0:G],
                                                     e[:, c0:G])
                            nc.tensor.matmul(o_ps[:], vnat[:, j, :], e[:],
                                             start=(j == 0), stop=(j == jn - 1))
                            if j % 2 == 1:
                                fill(1)
                        pending.append((h, g, o_ps, sumacc))
                    flush_tail()
                    fill(len(mq))

                flush_tail()

                # ---- out-proj for the last column
                for m in range(KT):
                    m_item(NG - 1, m)
    return nc


_NC_CACHE = None


def _get_nc():
    global _NC_CACHE
    if _NC_CACHE is None:
        nc = bacc.Bacc("TRN2", target_bir_lowering=False, debug=False,
                       num_devices=NCORES)
        _emit(nc)
        nc.compile()
        _NC_CACHE = nc
    return _NC_CACHE


def _install_ntff_hook():
    import sys
    import types
    try:
        import trn_agent_boot.trn_boot as tb
        hook = tb._ntff_profile_via_ctypes('/opt/axon/libaxon_pjrt.so')
        if hook is None:
            return
        mod = types.ModuleType('antenv.axon_hooks')
        mod.get_axon_ntff_profile_hook = lambda: hook
        sys.modules['antenv.axon_hooks'] = mod
    except Exception:
        pass


def kernel(**inputs):
    global LAST_EXEC_NS
    import ml_dtypes
    BF = ml_dtypes.bfloat16

    positions = np.asarray(inputs["positions"]).astype(np.float32)
    hidden = np.asarray(inputs["hidden_states"], dtype=np.float32)
    Wq = np.asarray(inputs["Wq"], dtype=np.float32)
    Wk = np.asarray(inputs["Wk"], dtype=np.float32)
    Wv = np.asarray(inputs["Wv"], dtype=np.float32)
    Wo = np.asarray(inputs["Wo"], dtype=np.float32)

    # RoPE tables, [128, T]: row p uses frequency p mod 64; rows 0-63 of
    # sind carry -sin (rotate-half low half), rows 64-127 carry +sin.
    half = D // 2
    inv = (1.0 / (1e6 ** (np.arange(half, dtype=np.float32) / half)))
    ang = (positions[:, None] * inv[None, :]).astype(np.float64)  # [T, 64]
    c = np.cos(ang).T  # [64, T]
    s = np.sin(ang).T
    cosd = np.concatenate([c, c], axis=0).astype(BF)
    sind = np.concatenate([-s, s], axis=0).astype(BF)

    # hidden^T tiles: hst[s, k, p, c] = hs[s*512 + c, k*128 + p]
    hst = np.ascontiguousarray(
        hidden.astype(BF).reshape(NG, G, KT, 128).transpose(0, 2, 3, 1))

    Wq_b = Wq.astype(BF)
    Wk_b = Wk.astype(BF)
    Wv_b = Wv.astype(BF)
    Wo_b = Wo.astype(BF)

    trace = os.environ.get("KERNEL_TRACE", "0") == "1"
    if trace:
        _install_ntff_hook()

    nc = _get_nc()
    in_maps = []
    for c_ in range(NCORES):
        wq_c = np.ascontiguousarray(
            Wq_b[:, c_ * DQ:(c_ + 1) * DQ].reshape(KT, 128, DQ)
            .transpose(1, 0, 2))
        wk_c = np.ascontiguousarray(
            Wk_b[:, c_ * D:(c_ + 1) * D].reshape(KT, 128, D)
            .transpose(1, 0, 2))
        wv_c = np.ascontiguousarray(
            Wv_b[:, c_ * D:(c_ + 1) * D].reshape(KT, 128, D)
            .transpose(1, 0, 2))
        wo_c = np.ascontiguousarray(
            Wo_b[c_ * DQ:(c_ + 1) * DQ, :].reshape(NH, 128, HID)
            .transpose(1, 0, 2))
        in_maps.append({
            "hst": hst,
            "wq": wq_c,
            "wk": wk_c,
            "wv": wv_c,
            "wo": wo_c,
            "cosd": cosd,
            "sind": sind,
        })
    res = run_bass_kernel_spmd(nc, in_maps, core_ids=list(range(NCORES)),
                               trace=trace)
    LAST_EXEC_NS = res.exec_time_ns
    acc = np.zeros((HID, T), dtype=np.float32)
    for c_ in range(NCORES):
        part = np.asarray(res.results[c_]["yt"]).astype(np.float32)
        acc += part.transpose(0, 2, 1, 3).reshape(HID, T)
    return np.ascontiguousarray(acc.T).astype(np.float32)


# revision 20
# speedup vs baseline: 1.0075x; 1.0075x over previous
"""Trainium2 Bass kernel for Mixtral-style GQA attention (v2, bf16).

Full module: y = Attn(RoPE(hs@Wq), RoPE(hs@Wk), hs@Wv) @ Wo
  T=2048, HIDDEN=4096, 32 Q heads / 8 KV heads, head_dim=128, causal,
  neox rotate-half RoPE (base 1e6), fp32 in/out.

Sharding (8 cores, tensor-parallel over heads):
  core c: Q heads 4c..4c+3 (Wq cols c*512:+512), KV head c (Wk/Wv cols
  c*128:+128), Wo rows c*512:+512.  Each core computes a partial
  y^T [4096, 2048] (bf16); host sums the 8 partials and transposes.

v2 design (vs v1): everything bf16 on the PE; hidden_states transposed
and weights re-laid-out on the HOST so the device does zero transposes
and all DMA is contiguous; RoPE sin/cos tables precomputed on host; V
projected directly into natural [t,d] layout; causal diagonal S blocks
column-trimmed; one continuous PE stream: attention for q-column 0 is
pre-computed inside the last projection group, and out-projection of
column g-1 is interleaved into attention column g as latency filler.
"""
import math
import os

import numpy as np

import concourse.bass as bass
import concourse.mybir as mybir
import concourse.tile as tile
from concourse import bacc
from concourse.bass_utils import run_bass_kernel_spmd

F32 = mybir.dt.float32
F32R = mybir.dt.float32r
BF16 = mybir.dt.bfloat16
I32 = mybir.dt.int32
AF = mybir.ActivationFunctionType
ALU = mybir.AluOpType

T = 2048
HID = 4096
NH = 4            # q heads per core
D = 128           # head dim
DQ = NH * D       # 512
G = 512           # seq group size
NG = T // G       # 4
KT = HID // 128   # 32 hidden k-tiles
NJ = T // 128     # 16 key blocks
NCORES = 8

SCALE = 1.0 / math.sqrt(D)

LAST_EXEC_NS = None


def _emit(nc):
    # host-prepared layouts (see kernel() below)
    hst = nc.dram_tensor("hst", [NG, KT, 128, G], BF16, kind="ExternalInput").ap()
    wq = nc.dram_tensor("wq", [128, KT, DQ], BF16, kind="ExternalInput").ap()
    wk = nc.dram_tensor("wk", [128, KT, D], BF16, kind="ExternalInput").ap()
    wv = nc.dram_tensor("wv", [128, KT, D], BF16, kind="ExternalInput").ap()
    wo = nc.dram_tensor("wo", [128, NH, HID], BF16, kind="ExternalInput").ap()
    cosd = nc.dram_tensor("cosd", [128, T], BF16, kind="ExternalInput").ap()
    sind = nc.dram_tensor("sind", [128, T], BF16, kind="ExternalInput").ap()
    yt = nc.dram_tensor("yt", [KT, NG, 128, G], BF16, kind="ExternalOutput").ap()

    with tile.TileContext(nc) as tc:
        with (
            tc.tile_pool(name="const", bufs=1) as const,
            tc.tile_pool(name="res", bufs=1) as res,
            tc.tile_pool(name="hp", bufs=8) as hp,
            tc.tile_pool(name="ro", bufs=2) as ro,
            tc.tile_pool(name="e0p", bufs=16) as e0p,
            tc.tile_pool(name="ex", bufs=6) as ex,
            tc.tile_pool(name="sc", bufs=2) as sc,
            tc.tile_pool(name="yo", bufs=4) as yo,
        ):
            # ---------------- constants ----------------
            onesf = const.tile([128, 1], F32, name="onesf", tag="onesf")
            nc.gpsimd.memset(onesf[:], 1.0)
            ones = const.tile([128, 1], F32R, name="ones", tag="ones")
            nc.scalar.copy(ones[:], onesf[:])

            # resident weights / tables / activations
            wq_sb = res.tile([128, KT, DQ], BF16, name="wq_sb", tag="wq_sb")
            wk_sb = res.tile([128, KT, D], BF16, name="wk_sb", tag="wk_sb")
            wv_sb = res.tile([128, KT, D], BF16, name="wv_sb", tag="wv_sb")
            wo_sb = res.tile([128, NH, HID], BF16, name="wo_sb", tag="wo_sb")
            cosf = res.tile([128, T], BF16, name="cosf", tag="cosf")
            sinpm = res.tile([128, T], BF16, name="sinpm", tag="sinpm")
            qt = [res.tile([128, T], BF16, name=f"qt{h}", tag=f"qt{h}")
                  for h in range(NH)]
            kt = res.tile([128, T], BF16, name="kt", tag="kt")
            vnat = res.tile([128, NJ, D], BF16, name="vnat", tag="vnat")

            # pre-computed exp tiles for attention column g=0 (filled
            # during projection group s=3, consumed right after)
            a0_es = [None] * (NH * 4)

            def rope(src_ps, dst, ssl, use_dve):
                raw = ro.tile([128, G], BF16, name="raw", tag="raw", bufs=3)
                if use_dve:
                    nc.vector.tensor_copy(raw[:], src_ps[:])
                else:
                    nc.scalar.copy(raw[:], src_ps[:])
                rot = ro.tile([128, G], BF16, name="rot", tag="rot", bufs=3)
                nc.scalar.dma_start(rot[0:64, :], raw[64:128, :])
                nc.scalar.dma_start(rot[64:128, :], raw[0:64, :])
                tmp = ro.tile([128, G], BF16, name="tmp", tag="tmp", bufs=2)
                nc.vector.tensor_mul(tmp[:], rot[:], sinpm[:, ssl])
                nc.vector.tensor_mul(dst, raw[:], cosf[:, ssl])
                nc.vector.tensor_add(dst, dst, tmp[:])

            def emit_exp(e, s_ps, g, j):
                """exp + causal handling for S^T block j of column g.
                Writes only e[:, c0:]; cols [0, c0) are never read (sum
                adds and PV are sliced to [c0:] as well)."""
                r = j - 4 * g
                c0 = 128 * r if r > 0 else 0
                nc.scalar.activation(e[:, c0:G], s_ps[:, c0:G], AF.Exp,
                                     scale=SCALE)
                if r >= 0:
                    nc.gpsimd.affine_select(
                        out=e[:, c0:G], in_=e[:, c0:G],
                        compare_op=ALU.is_ge, fill=0.0,
                        base=0, channel_multiplier=-1,
                        pattern=[[1, G - c0]])
                return c0

            # ---------------- phase P: projections ----------------
            with (
                tc.tile_pool(name="accp", bufs=6, space="PSUM") as accp,
                tc.tile_pool(name="pre", bufs=2, space="PSUM") as pre,
            ):
                for s in range(NG):
                    ssl = bass.ts(s, G)
                    q_ps = [accp.tile([128, G], F32, name=f"qps{f}",
                                      tag="acc") for f in range(NH)]
                    k_ps = accp.tile([128, G], F32, name="kps", tag="acc")
                    v_ps = accp.tile([128, NH, D], F32, name="vps", tag="acc")

                    for k in range(KT):
                        ht = hp.tile([128, G], BF16, name="ht", tag="ht")
                        nc.sync.dma_start(ht[:], hst[s, k])
                        # stream weights during s=0, misc during s=1;
                        # first chunk is small so the PE starts sooner
                        if s == 0:
                            chunks = {0: (0, 2), 2: (2, 8), 10: (10, 8),
                                      18: (18, 8), 26: (26, 6)}
                            if k in chunks:
                                ck = bass.ds(*chunks[k])
                                nc.sync.dma_start(wq_sb[:, ck, :], wq[:, ck, :])
                                nc.sync.dma_start(wk_sb[:, ck, :], wk[:, ck, :])
                                nc.sync.dma_start(wv_sb[:, ck, :], wv[:, ck, :])
                            if k == 9:
                                nc.sync.dma_start(cosf[:], cosd)
                            if k == 11:
                                nc.sync.dma_start(sinpm[:], sind)
                        elif s == 1 and k < NH:
                            nc.sync.dma_start(wo_sb[:, k, :], wo[:, k, :])
                        st = (k == 0)
                        sp = (k == KT - 1)
                        for f in range(NH):
                            nc.tensor.matmul(
                                q_ps[f][:], wq_sb[:, k, f * 128:(f + 1) * 128],
                                ht[:], start=st, stop=sp)
                        nc.tensor.matmul(k_ps[:], wk_sb[:, k, :], ht[:],
                                         start=st, stop=sp)
                        # all 4 tb regions live in ONE 2KB psum bank and
                        # start=True zeroes the whole bank: only the first
                        # write starts it, only the last one stops it.
                        for tb in range(NH):
                            nc.tensor.matmul(
                                v_ps[:, tb, :], ht[:, tb * 128:(tb + 1) * 128],
                                wv_sb[:, k, :], start=(st and tb == 0),
                                stop=(sp and tb == NH - 1))

                        # pre-compute attention for q-column 0 during s=3
                        if s == 3 and k >= KT - 16:
                            i = k - (KT - 16)
                            h, r = divmod(i, 4)
                            s_ps = pre.tile([128, G], F32, name="sps",
                                            tag="sps")
                            c0 = 128 * r
                            nc.tensor.matmul(
                                s_ps[:, c0:G], kt[:, r * 128:(r + 1) * 128],
                                qt[h][:, c0:G], start=True, stop=True)
                            e = e0p.tile([128, G], BF16, name="e0", tag="e0")
                            emit_exp(e, s_ps, 0, r)
                            a0_es[i] = e

                    # epilogue: RoPE for q heads + k; v to natural layout
                    for f in range(NH):
                        rope(q_ps[f], qt[f][:, ssl], ssl, use_dve=(f % 2 == 1))
                    rope(k_ps, kt[:, ssl], ssl, use_dve=False)
                    nc.vector.tensor_copy(vnat[:, NH * s:NH * s + NH, :],
                                          v_ps[:])

            # ---------------- phase A + O: attention & out-proj ----------
            with (
                tc.tile_pool(name="pss", bufs=4, space="PSUM") as pss,
                tc.tile_pool(name="pso", bufs=2, space="PSUM") as pso,
                tc.tile_pool(name="pssum", bufs=1, space="PSUM") as pssum,
                tc.tile_pool(name="psy", bufs=1, space="PSUM") as psy,
            ):
                def head_tail(h, g, o_ps, sumacc):
                    """softmax-normalize accumulated PV -> qt[h] (as O^T)."""
                    gsl = bass.ts(g, G)
                    s_sum = pssum.tile([1, G], F32, name="ssum", tag="ssum")
                    nc.tensor.matmul(s_sum[:], ones[:], sumacc[:],
                                     start=True, stop=True)
                    recrow = sc.tile([1, G], F32, name="recrow", tag="recrow")
                    nc.vector.reciprocal_approx_fast(recrow[:], s_sum[:])
                    recb = sc.tile([128, G], F32, name="recb", tag="recb")
                    nc.gpsimd.partition_broadcast(recb[:], recrow[:])
                    nc.vector.tensor_mul(qt[h][:, gsl], o_ps[:], recb[:])

                def m_item(g_out, m, pool, act_copy=None):
                    """out-projection tile m for q-column g_out."""
                    gsl = bass.ts(g_out, G)
                    y_ps = pool.tile([128, G], F32, name="yps", tag="yps")
                    for f in range(NH):
                        nc.tensor.matmul(y_ps[:], wo_sb[:, f, m * 128:(m + 1) * 128],
                                         qt[f][:, gsl],
                                         start=(f == 0), stop=(f == NH - 1))
                    y_sb = yo.tile([128, G], BF16, name="ysb", tag="ysb")
                    on_act = (m % 2 == 1) if act_copy is None else act_copy
                    if on_act:
                        nc.scalar.copy(y_sb[:], y_ps[:])
                    else:
                        nc.vector.tensor_copy(y_sb[:], y_ps[:])
                    nc.sync.dma_start(yt[m, g_out], y_sb[:])

                # deferred softmax-tail: emit each head's tail after the
                # next head's PE stream is primed, so the tiny ssum matmul
                # (which waits on the DVE sum chain) never stalls the PE.
                pending = []

                def flush_tail():
                    while pending:
                        head_tail(*pending.pop(0))

                # ---- column 0: PV of the pre-computed exps (pure stream)
                for h in range(NH):
                    sumacc = sc.tile([128, G], F32R, name="sumacc",
                                     tag="sumacc", bufs=2)
                    o_ps = pso.tile([128, G], F32, name="ops", tag="ops")
                    for j in range(4):
                        e = a0_es[h * 4 + j]
                        c0 = 128 * j
                        if j == 0:
                            nc.vector.tensor_copy(sumacc[:], e[:])
                        else:
                            nc.vector.tensor_add(sumacc[:, c0:G],
                                                 sumacc[:, c0:G], e[:, c0:G])
                        # start=True zeroes the whole psum bank, so the
                        # sliced diagonal PVs accumulate onto zeroed cols
                        nc.tensor.matmul(o_ps[:, c0:G], vnat[:, j, :],
                                         e[:, c0:G],
                                         start=(j == 0), stop=(j == 3))
                        if j == 1:
                            flush_tail()
                    pending.append((h, 0, o_ps, sumacc))

                # ---- columns 1..3 with out-proj of g-1 as filler;
                # unconsumed filler spills into the next column (and the
                # final tail section) instead of bursting at column end
                mqueue = []

                def fill(n):
                    for _ in range(n):
                        if mqueue:
                            g_out, m = mqueue.pop(0)
                            m_item(g_out, m, psy)

                for g in range(1, NG):
                    jn = 4 * g + 4
                    mqueue.extend((g - 1, m) for m in range(KT))

                    for h in range(NH):
                        s_tiles = {}

                        def emit_s(j, h=h, g=g):
                            s_ps = pss.tile([128, G], F32, name="sps",
                                            tag="sps")
                            r = j - 4 * g
                            c0 = 128 * r if r > 0 else 0
                            nc.tensor.matmul(
                                s_ps[:, c0:G], kt[:, j * 128:(j + 1) * 128],
                                qt[h][:, bass.ds(g * G + c0, G - c0)],
                                start=True, stop=True)
                            s_tiles[j] = s_ps

                        for j in range(3):
                            emit_s(j)
                        if h == 0:
                            # pending tail writes qt[3] in column g-1, which
                            # the filler out-proj reads: flush it first
                            flush_tail()
                            fill(2)
                        else:
                            # pending tail writes the CURRENT column (not
                            # read by fillers): fill first for DVE slack
                            fill(2)
                            flush_tail()
                        sumacc = sc.tile([128, G], F32R, name="sumacc",
                                         tag="sumacc", bufs=2)
                        o_ps = pso.tile([128, G], F32, name="ops", tag="ops")
                        # spread 5 fill items evenly over the j loop so the
                        # PE always has exp-independent work queued
                        fill_at = set((jn * (i + 1)) // 6 for i in range(5))
                        for j in range(jn):
                            s_ps = s_tiles.pop(j)
                            e = ex.tile([128, G], BF16, name="esb", tag="esb")
                            c0 = emit_exp(e, s_ps, g, j)
                            if j + 3 < jn:
                                emit_s(j + 3)
                            if j == 0:
                                nc.vector.tensor_copy(sumacc[:], e[:])
                            else:
                                nc.vector.tensor_add(sumacc[:, c0:G],
                                                     sumacc[:, c0:G],
                                                     e[:, c0:G])
                            nc.tensor.matmul(o_ps[:, c0:G], vnat[:, j, :],
                                             e[:, c0:G],
                                             start=(j == 0), stop=(j == jn - 1))
                            if j in fill_at:
                                fill(1)
                        pending.append((h, g, o_ps, sumacc))

                # last head's tail must go out while its pools are open;
                # spilled fillers (independent of it) cover the sum-chain
                fill(2)
                flush_tail()
                fill(2)

            # ---- tail: remaining spill + out-proj of the last column.
            # attention pools are closed; use a deeper y-psum pool so the
            # back-to-back out-proj stream double-buffers cleanly. ACT is
            # idle here, so most psum->sbuf copies go to it.
            with tc.tile_pool(name="psy2", bufs=4, space="PSUM") as psy2:
                spill = list(mqueue)
                mqueue.clear()
                for i, (g_out, m) in enumerate(spill):
                    m_item(g_out, m, psy2, act_copy=(i % 4 != 0))
                for m in range(KT):
                    m_item(NG - 1, m, psy2, act_copy=(m % 4 != 0))
    return nc


_NC_CACHE = None


def _get_nc():
    global _NC_CACHE
    if _NC_CACHE is None:
        nc = bacc.Bacc("TRN2", target_bir_lowering=False, debug=False,
                       num_devices=NCORES)
        _emit(nc)
        nc.compile()
        _NC_CACHE = nc
    return _NC_CACHE


def _install_ntff_hook():
    import sys
    import types
    try:
        import trn_agent_boot.trn_boot as tb
        hook = tb._ntff_profile_via_ctypes('/opt/axon/libaxon_pjrt.so')
        if hook is None:
            return
        mod = types.ModuleType('antenv.axon_hooks')
        mod.get_axon_ntff_profile_hook = lambda: hook
        sys.modules['antenv.axon_hooks'] = mod
    except Exception:
        pass


def kernel(**inputs):
    global LAST_EXEC_NS
    import ml_dtypes
    BF = ml_dtypes.bfloat16

    positions = np.asarray(inputs["positions"]).astype(np.float32)
    hidden = np.asarray(inputs["hidden_states"], dtype=np.float32)
    Wq = np.asarray(inputs["Wq"], dtype=np.float32)
    Wk = np.asarray(inputs["Wk"], dtype=np.float32)
    Wv = np.asarray(inputs["Wv"], dtype=np.float32)
    Wo = np.asarray(inputs["Wo"], dtype=np.float32)

    # RoPE tables, [128, T]: row p uses frequency p mod 64; rows 0-63 of
    # sind carry -sin (rotate-half low half), rows 64-127 carry +sin.
    half = D // 2
    inv = (1.0 / (1e6 ** (np.arange(half, dtype=np.float32) / half)))
    ang = (positions[:, None] * inv[None, :]).astype(np.float64)  # [T, 64]
    c = np.cos(ang).T  # [64, T]
    s = np.sin(ang).T
    cosd = np.concatenate([c, c], axis=0).astype(BF)
    sind = np.concatenate([-s, s], axis=0).astype(BF)

    # hidden^T tiles: hst[s, k, p, c] = hs[s*512 + c, k*128 + p]
    hst = np.ascontiguousarray(
        hidden.astype(BF).reshape(NG, G, KT, 128).transpose(0, 2, 3, 1))

    Wq_b = Wq.astype(BF)
    Wk_b = Wk.astype(BF)
    Wv_b = Wv.astype(BF)
    Wo_b = Wo.astype(BF)

    trace = os.environ.get("KERNEL_TRACE", "0") == "1"
    if trace:
        _install_ntff_hook()

    nc = _get_nc()
    in_maps = []
    for c_ in range(NCORES):
        wq_c = np.ascontiguousarray(
            Wq_b[:, c_ * DQ:(c_ + 1) * DQ].reshape(KT, 128, DQ)
            .transpose(1, 0, 2))
        wk_c = np.ascontiguousarray(
            Wk_b[:, c_ * D:(c_ + 1) * D].reshape(KT, 128, D)
            .transpose(1, 0, 2))
        wv_c = np.ascontiguousarray(
            Wv_b[:, c_ * D:(c_ + 1) * D].reshape(KT, 128, D)
            .transpose(1, 0, 2))
        wo_c = np.ascontiguousarray(
            Wo_b[c_ * DQ:(c_ + 1) * DQ, :].reshape(NH, 128, HID)
            .transpose(1, 0, 2))
        in_maps.append({
            "hst": hst,
            "wq": wq_c,
            "wk": wk_c,
            "wv": wv_c,
            "wo": wo_c,
            "cosd": cosd,
            "sind": sind,
        })
    res = run_bass_kernel_spmd(nc, in_maps, core_ids=list(range(NCORES)),
                               trace=trace)
    LAST_EXEC_NS = res.exec_time_ns
    acc = np.zeros((HID, T), dtype=np.float32)
    for c_ in range(NCORES):
        part = np.asarray(res.results[c_]["yt"]).astype(np.float32)
        acc += part.transpose(0, 2, 1, 3).reshape(HID, T)
    return np.ascontiguousarray(acc.T).astype(np.float32)
